# revision 7
# baseline (speedup 1.0000x reference)
"""GAT (2-layer, 8-head) Trainium2 Bass kernel, SPMD over 8 NeuronCores.

Strategy (dst-node-parallel with replicated projection):
  - Host: sort edges by dst, give each core the edges whose dst falls in its
    1/8 node range. Precompute fused projection matrices Wcat = [W_heads | a_sj | a_si]
    (score halves pre-scaled x16 for fp8 range) so one matmul produces per-node
    head embeddings AND both attention score halves. Also precompute the
    per-chunk one-hot dst matrices and a pre-transposed copy of x.
  - Device, per core (single NEFF):
      Phase A: project ALL nodes (replicated) into an fp8 row table
               table[r(n)] = [h (1024 fp8) | sj x16 (8 fp8) | si x16 (8 fp8) | pad],
               row order r(n) permuted partition-major within 512-node blocks so
               the table store is one cheap SWDGE cast-DMA per block.
      Phase B: per owned 128-node tile: dma_gather rows by src (1280B) and
               score tails by dst (256B), leaky+exp (exp carries the 1/16
               descale), aggregate via onehot-matmul into PSUM (denominator =
               extra matmul columns), per-head normalize on ACT, head-mean+ELU.
      Phase C: AllGather emb1, SPLIT in two pipelined halves (tiles 0-4 ->
               AG-a issued mid-edge-phase, tiles 5-9 -> AG-b) with shards
               padded to 640 rows so layer-2 projection blocks stay 512-row
               aligned. AG-a overlaps the tail of the edge phase; AG-b
               overlaps layer-2 projection of the first half.
      Phase D/E: same for layer 2 -> per-core partial graph-sum [1, 128].
  - Host: sum partials, mean, LayerNorm + 3-layer MLP head -> [16].
"""

import numpy as np
import ml_dtypes

BF16 = ml_dtypes.bfloat16

# problem sizes (fixed for this problem)
N_NODES = 10000
N_EDGES = 160000
N_FEAT = 512
HEADS = 8
HID = 128
OUT = 16

N_CORES = 8
P = 128

SCORE_SCALE = 16.0  # sj/si stored x16 so fp8e4m3 keeps precision

NLOC = N_NODES // N_CORES   # 1250
HALF = 640                  # padded per-core shard rows per AllGather half
NP2 = N_CORES * HALF        # 5120 rows per gathered half (10 blocks of 512)


# ----------------------------------------------------------------------------
# host-side prep
# ----------------------------------------------------------------------------

def _wrap_idx(idx_i16: np.ndarray) -> np.ndarray:
    """[n] int16 -> [128, n//16] wrapped layout for dma_gather (16-partition
    wrap, replicated 8x down the partitions)."""
    n = idx_i16.shape[0]
    assert n % 16 == 0
    w = idx_i16.reshape(n // 16, 16).T  # [16, n//16]
    return np.tile(w, (8, 1))  # [128, n//16]


def _perm(n: np.ndarray) -> np.ndarray:
    """Row remap: node n -> table row. Within each 512-node block, rows are
    stored partition-major ((n%512)%128 outer, (n%512)//128 inner) so the
    projection's [P, SB, cols] SBUF tile lands contiguously per partition."""
    blk = n // 512
    off = n % 512
    return blk * 512 + (off % 128) * 4 + off // 128


def _pad2(n: np.ndarray) -> np.ndarray:
    """Node id -> padded layer-2 row id (per-core shards padded to 2x HALF,
    first halves concatenated then second halves)."""
    c = n // NLOC
    off = n % NLOC
    return np.where(off < HALF, c * HALF + off,
                    NP2 + c * HALF + (off - HALF))


def host_prep(node_features, edge_src, edge_dst, W1, a1, W2, a2):
    """Returns (in_maps, meta). meta carries static structure for the builder."""
    nloc = NLOC
    T = -(-nloc // P)  # owned tiles per core (10)

    order = np.argsort(edge_dst, kind="stable")
    src_s = edge_src[order].astype(np.int64)
    dst_s = edge_dst[order].astype(np.int64)

    # common chunk count per tile position (same across cores; program is SPMD)
    tile_cnt = np.zeros((N_CORES, T), dtype=np.int64)
    for c in range(N_CORES):
        base = c * nloc
        for t in range(T):
            n0 = base + t * P
            n1 = min(base + (t + 1) * P, (c + 1) * nloc)
            lo = np.searchsorted(dst_s, n0, side="left")
            hi = np.searchsorted(dst_s, n1, side="left")
            tile_cnt[c, t] = hi - lo
    C_t = [int(-(-tile_cnt[:, t].max() // P)) for t in range(T)]  # chunks per tile pos

    SLOTS = [c * P for c in C_t]
    tot_chunks = sum(C_t)
    tot_slots = tot_chunks * P

    in_maps = []
    xtv = np.ascontiguousarray(node_features.astype(BF16).T)  # [512, 10000]

    # fused projection matrices; score halves pre-scaled for fp8 storage
    def wcat(W, a, K):
        w = np.transpose(W, (2, 0, 1)).reshape(K, HEADS * HID)  # [K, H*D]
        si = np.einsum("hdf,hd->fh", W, a[:, :HID]) * SCORE_SCALE  # [K, H] dst-side
        sj = np.einsum("hdf,hd->fh", W, a[:, HID:]) * SCORE_SCALE  # [K, H] src-side
        return np.concatenate([w, sj, si], axis=1).astype(BF16)  # [K, H*D+16]

    w1c = np.ascontiguousarray(wcat(W1, a1, N_FEAT))
    w2c = np.ascontiguousarray(wcat(W2, a2, HID))

    def widx(ids, t_ranges):
        return np.concatenate(
            [_wrap_idx(ids[a:b].astype(np.int16)) for (a, b) in t_ranges], axis=1)

    for c in range(N_CORES):
        base = c * nloc
        src_pad = np.zeros(tot_slots, dtype=np.int64)
        dst_pad = np.zeros(tot_slots, dtype=np.int64)
        dstl_pad = np.full(tot_slots, 30000.0, dtype=np.float32)
        off = 0
        for t in range(T):
            n0 = base + t * P
            n1 = min(base + (t + 1) * P, (c + 1) * nloc)
            lo = np.searchsorted(dst_s, n0, side="left")
            hi = np.searchsorted(dst_s, n1, side="left")
            k = hi - lo
            sl = SLOTS[t]
            src_pad[off:off + k] = src_s[lo:hi]
            dst_pad[off:off + k] = dst_s[lo:hi]
            dstl_pad[off:off + k] = (dst_s[lo:hi] - n0).astype(np.float32)
            # padding: src/dst index 0 (valid gather), dstl 30000 (never matches)
            off += sl
        assert off == tot_slots

        t_ranges = [(sum(SLOTS[:t]), sum(SLOTS[:t + 1])) for t in range(T)]
        # layer-1 gather rows: perm over raw node ids
        src16_1 = widx(_perm(src_pad), t_ranges)
        dst16_1 = widx(_perm(dst_pad), t_ranges)
        # layer-2 gather rows: perm over padded shard ids
        src16_2 = widx(_perm(_pad2(src_pad)), t_ranges)
        dst16_2 = widx(_perm(_pad2(dst_pad)), t_ranges)

        # one-hot dst matrices, built on host: ohm[p, g*128+d] = (dstl[g*128+p]==d)
        dstl_cols = dstl_pad.reshape(tot_chunks, P)  # [chunk, edge-in-chunk]
        ohm = (dstl_cols[:, :, None] == np.arange(P)[None, None, :])  # [g, p, d]
        ohm = np.ascontiguousarray(
            ohm.transpose(1, 0, 2).reshape(P, tot_chunks * P).astype(BF16))

        in_maps.append({
            "xtv": xtv,
            "w1c": w1c,
            "w2c": w2c,
            "src16_1": np.ascontiguousarray(src16_1),
            "dst16_1": np.ascontiguousarray(dst16_1),
            "src16_2": np.ascontiguousarray(src16_2),
            "dst16_2": np.ascontiguousarray(dst16_2),
            "ohm": ohm,
        })

    meta = {
        "T": T, "C_t": C_t, "nloc": nloc,
        "tot_chunks": tot_chunks,
    }
    return in_maps, meta


# ----------------------------------------------------------------------------
# device program
# ----------------------------------------------------------------------------

HD = HEADS * HID          # 1024
ROW = 1280                # fp8 bytes per table row (1040 used, %256)
PCOLS = HD + 16           # projection output cols: [h (1024) | sj 8 | si 8]
TAIL = 256                # score-tail gather bytes (row bytes 1024:1280)


def build_program(meta, debug=False, stages=5, iters=1, ablate=""):
    import concourse.bacc as bacc
    import concourse.mybir as mybir
    import concourse.tile as tile
    from concourse.library_config import mlp

    dt = mybir.dt
    Alu = mybir.AluOpType
    Act = mybir.ActivationFunctionType

    T = meta["T"]
    C_t = meta["C_t"]
    nloc = meta["nloc"]
    tot_chunks = meta["tot_chunks"]

    K1 = N_FEAT // P       # 4 contraction chunks layer 1
    RB = 512               # rows per projection block

    nc = bacc.Bacc("TRN2", num_devices=N_CORES, num_swdge_queues=2,
                   dynamic_dma_scratch_size=49152)

    xtv = nc.dram_tensor("xtv", [N_FEAT, N_NODES], dt.bfloat16, kind="ExternalInput")
    w1c = nc.dram_tensor("w1c", [N_FEAT, PCOLS], dt.bfloat16, kind="ExternalInput")
    w2c = nc.dram_tensor("w2c", [HID, PCOLS], dt.bfloat16, kind="ExternalInput")
    idx_in = {}
    for nm in ("src16_1", "dst16_1", "src16_2", "dst16_2"):
        idx_in[nm] = nc.dram_tensor(nm, [P, tot_chunks * 8], dt.int16,
                                    kind="ExternalInput")
    ohm = nc.dram_tensor("ohm", [P, tot_chunks * P], dt.bfloat16, kind="ExternalInput")

    out_vec = nc.dram_tensor("out_vec", [1, HID], dt.float32, kind="ExternalOutput")

    NPAD1 = (-(-N_NODES // RB)) * RB  # 10240
    table1 = nc.dram_tensor("table1", [NPAD1, ROW], dt.float8e4)
    table2 = nc.dram_tensor("table2", [2 * NP2, ROW], dt.float8e4)
    emb_la = nc.dram_tensor("emb_la", [HALF, HID], dt.bfloat16)
    emb_lb = nc.dram_tensor("emb_lb", [HALF, HID], dt.bfloat16)
    emb_ga = nc.dram_tensor("emb_ga", [NP2, HID], dt.bfloat16, addr_space="Shared")
    emb_gb = nc.dram_tensor("emb_gb", [NP2, HID], dt.bfloat16, addr_space="Shared")

    with tile.TileContext(nc) as tc:
        with (
            tc.tile_pool(name="const", bufs=1) as cpool,
            tc.tile_pool(name="xtp", bufs=8) as tpool,
            tc.tile_pool(name="work", bufs=2) as wpool,
            tc.tile_pool(name="chunk", bufs=3) as kpool,
            tc.tile_pool(name="psum", bufs=2, space="PSUM") as pspool,
            tc.tile_pool(name="psg", bufs=1, space="PSUM") as psg,
        ):
            nc.gpsimd.load_library(mlp)

            # ---- constants ----
            ones_col = cpool.tile([P, 1], dt.bfloat16)
            nc.gpsimd.memset(ones_col[:], 1.0)

            w1s = cpool.tile([P, K1 * PCOLS], dt.bfloat16)
            nc.sync.dma_start(
                out=w1s[:].rearrange("p (k c) -> p k c", k=K1),
                in_=w1c[:].rearrange("(k p) c -> p k c", p=P))
            w2s = cpool.tile([P, PCOLS], dt.bfloat16)
            nc.sync.dma_start(out=w2s[:], in_=w2c[:])

            # per-layer gather indices share one SBUF tile pair (reloaded
            # between layers; WAR/RAW ordering handled by the tile framework)
            srcI = cpool.tile([P, tot_chunks * 8], dt.int16)
            dstI = cpool.tile([P, tot_chunks * 8], dt.int16)
            nc.sync.dma_start(out=srcI[:], in_=idx_in["src16_1"][:])
            nc.sync.dma_start(out=dstI[:], in_=idx_in["dst16_1"][:])
            ohS = cpool.tile([P, tot_chunks * P], dt.bfloat16)
            nc.sync.dma_start(out=ohS[:], in_=ohm[:])

            # ---------------- projection phase ----------------
            def project(src_dram, wtile, K, table, row0, nrows, pre_t):
                """table[row0 + r(n)] = [h | sj | si] (fp8, x16 scores).
                pre_t: src_dram is already transposed ([K*P, nrows])."""
                nblk = -(-nrows // RB)
                for b in range(nblk):
                    r0 = b * RB
                    rn = min(RB, nrows - r0)
                    xT = []
                    for k in range(K):
                        t_ = tpool.tile([P, RB], dt.bfloat16, tag="xT")
                        eng = nc.sync if k % 2 == 0 else nc.scalar
                        if pre_t:
                            eng.dma_start(
                                out=t_[:, :rn],
                                in_=src_dram[k * P:(k + 1) * P, r0:r0 + rn])
                        else:
                            eng.dma_start_transpose(
                                t_[:, :rn], src_dram[r0:r0 + rn, k * P:(k + 1) * P])
                        xT.append(t_)
                    SB = RB // P
                    row = wpool.tile([P, SB, ROW], dt.bfloat16, tag="row")
                    for s in range(SB):
                        n0 = r0 + s * P
                        if n0 >= nrows:
                            break
                        nn = min(P, nrows - n0)
                        ps = pspool.tile([P, PCOLS], dt.float32, tag="ps")
                        for k in range(K):
                            lhsT = xT[k][:, s * P:s * P + nn]
                            rhs = wtile[:, k * PCOLS:(k + 1) * PCOLS] if K > 1 \
                                else wtile[:]
                            st, sp = (k == 0), (k == K - 1)
                            nc.tensor.matmul(ps[:nn, 0:512], lhsT=lhsT,
                                             rhs=rhs[:, 0:512], start=st, stop=sp)
                            nc.tensor.matmul(ps[:nn, 512:1024], lhsT=lhsT,
                                             rhs=rhs[:, 512:1024], start=st, stop=sp)
                            nc.tensor.matmul(ps[:nn, 1024:PCOLS], lhsT=lhsT,
                                             rhs=rhs[:, 1024:PCOLS], start=st, stop=sp)
                        # drain split across DVE and ACT
                        nc.vector.tensor_copy(row[:nn, s, 0:512], ps[:nn, 0:512])
                        nc.scalar.activation(row[:nn, s, 512:PCOLS],
                                             ps[:nn, 512:PCOLS], Act.Copy)
                    # one SWDGE cast-DMA (bf16->fp8): rows partition-major.
                    # Tail block writes junk rows into the table pad — never
                    # referenced by any gather index.
                    nc.gpsimd.dma_start(
                        out=table[row0 + r0:row0 + r0 + RB, :].rearrange(
                            "(p s) c -> p s c", p=P),
                        in_=row[:])

            # ---------------- edge phase ----------------
            def edges(table, layer):
                gps = psg.tile([1, HID], dt.float32, tag="gsum", name="gps") if layer == 2 else None
                ebuf = cpool.tile([P, T * HID], dt.bfloat16, name="ebuf") if layer == 2 else None
                for t in range(T):
                    C = C_t[t]
                    ioff = sum(C_t[:t])
                    n0t = t * P
                    nn_t = min(P, nloc - n0t)
                    G = wpool.tile([P, C, ROW], dt.float8e4, tag="G")
                    SIG = wpool.tile([P, C, TAIL], dt.float8e4, tag="SIG")
                    Ch = C // 2
                    nc.gpsimd.dma_gather(
                        G[:, 0:Ch, :], table[:], srcI[:, ioff * 8:(ioff + Ch) * 8],
                        Ch * P, Ch * P, ROW, single_packet=False, queue_num=0)
                    nc.gpsimd.dma_gather(
                        G[:, Ch:C, :], table[:],
                        srcI[:, (ioff + Ch) * 8:(ioff + C) * 8],
                        (C - Ch) * P, (C - Ch) * P, ROW, single_packet=False,
                        queue_num=1)
                    nc.gpsimd.dma_gather(
                        SIG[:, 0:Ch, :], table[:, 1024:ROW],
                        dstI[:, ioff * 8:(ioff + Ch) * 8],
                        Ch * P, Ch * P, TAIL, elem_step=ROW,
                        single_packet=False, queue_num=1)
                    nc.gpsimd.dma_gather(
                        SIG[:, Ch:C, :], table[:, 1024:ROW],
                        dstI[:, (ioff + Ch) * 8:(ioff + C) * 8],
                        (C - Ch) * P, (C - Ch) * P, TAIL, elem_step=ROW,
                        single_packet=False, queue_num=0)
                    ps = pspool.tile([P, PCOLS], dt.float32, tag="ps")
                    # batched attention logits for the whole tile (x16 scale)
                    LG = wpool.tile([P, C * 8], dt.float32, tag="LG")
                    nc.vector.tensor_tensor(
                        out=LG[:].rearrange("p (c e) -> p c e", c=C),
                        in0=SIG[:, :, 8:16], in1=G[:, :, 1024:1032], op=Alu.add)
                    # leaky_relu on DVE (still x16), exp on ACT with scale 1/16
                    LR = wpool.tile([P, C * 8], dt.float32, tag="LR")
                    nc.vector.tensor_scalar_mul(LR[:], LG[:], 0.01)
                    MX = wpool.tile([P, C * 8], dt.float32, tag="MX")
                    nc.vector.tensor_tensor(out=MX[:], in0=LG[:], in1=LR[:],
                                            op=Alu.max)
                    EX = wpool.tile([P, C * 8], dt.float32, tag="EX")
                    nc.scalar.activation(EX[:], MX[:], Act.Exp,
                                         scale=1.0 / SCORE_SCALE)
                    EXbf = wpool.tile([P, C, 8], dt.bfloat16, tag="EXbf")
                    nc.vector.tensor_copy(
                        EXbf[:], EX[:].rearrange("p (c e) -> p c e", c=C))
                    for c in range(C):
                        msg = kpool.tile([P, HD], dt.bfloat16, tag="msg")
                        for h in range(HEADS):
                            if h in (3, 7):
                                nc.scalar.activation(
                                    msg[:, h * HID:(h + 1) * HID],
                                    G[:, c, h * HID:(h + 1) * HID],
                                    Act.Copy,
                                    scale=EX[:, c * 8 + h:c * 8 + h + 1])
                            else:
                                nc.vector.tensor_scalar_mul(
                                    msg[:, h * HID:(h + 1) * HID],
                                    G[:, c, h * HID:(h + 1) * HID],
                                    EX[:, c * 8 + h:c * 8 + h + 1])
                        lhs = ohS[:, (ioff + c) * P:(ioff + c + 1) * P]
                        st, sp = (c == 0), (c == C - 1)
                        nc.tensor.matmul(ps[:, 0:512], lhsT=lhs,
                                         rhs=msg[:, 0:512], start=st, stop=sp)
                        nc.tensor.matmul(ps[:, 512:1024], lhsT=lhs,
                                         rhs=msg[:, 512:1024], start=st, stop=sp)
                        nc.tensor.matmul(ps[:, 1024:1032], lhsT=lhs,
                                         rhs=EXbf[:, c, :], start=st, stop=sp)
                    # ---- postprocess tile ----
                    den = kpool.tile([P, 8], dt.float32, tag="den")
                    nc.vector.tensor_scalar(out=den[:], in0=ps[:, 1024:1032],
                                            scalar1=float(HEADS), scalar2=1e-30,
                                            op0=Alu.mult, op1=Alu.max)
                    rec = kpool.tile([P, 8], dt.float32, tag="rec")
                    nc.vector.reciprocal(rec[:], den[:])
                    # per-head normalize on ACT (PSUM src), tree-add on DVE
                    hn = kpool.tile([P, HEADS, HID], dt.bfloat16, tag="hn")
                    for h in range(HEADS):
                        nc.scalar.activation(hn[:, h, :],
                                             ps[:, h * HID:(h + 1) * HID],
                                             Act.Copy, scale=rec[:, h:h + 1])
                    s4a = kpool.tile([P, 4, HID], dt.bfloat16, tag="s4a")
                    nc.vector.tensor_tensor(out=s4a[:], in0=hn[:, 0:4, :],
                                            in1=hn[:, 4:8, :], op=Alu.add)
                    s2a = kpool.tile([P, 2, HID], dt.bfloat16, tag="s2a")
                    nc.vector.tensor_tensor(out=s2a[:], in0=s4a[:, 0:2, :],
                                            in1=s4a[:, 2:4, :], op=Alu.add)
                    S = kpool.tile([P, HID], dt.float32, tag="S")
                    nc.vector.tensor_tensor(out=S[:], in0=s2a[:, 0, :],
                                            in1=s2a[:, 1, :], op=Alu.add)
                    # elu(S) = exp(min(S,0)) - 1 + max(S,0)
                    neg = kpool.tile([P, HID], dt.float32, tag="neg")
                    nc.vector.tensor_scalar_min(neg[:], S[:], 0.0)
                    en = kpool.tile([P, HID], dt.float32, tag="en")
                    nc.scalar.activation(en[:], neg[:], Act.Exp)
                    pos = kpool.tile([P, HID], dt.float32, tag="pos")
                    nc.vector.tensor_scalar_max(pos[:], S[:], 0.0)
                    eadd = kpool.tile([P, HID], dt.float32, tag="eadd")
                    nc.vector.tensor_tensor(out=eadd[:], in0=en[:], in1=pos[:],
                                            op=Alu.add)
                    if layer == 1:
                        ebf = kpool.tile([P, HID], dt.bfloat16, tag="ebf")
                        nc.vector.tensor_scalar_add(ebf[:], eadd[:], -1.0)
                        if t < 5:
                            nc.sync.dma_start(
                                out=emb_la[n0t:n0t + nn_t, :], in_=ebf[:nn_t, :])
                            if t == 4 and stages >= 3:
                                # first-half AllGather overlaps tiles 5-9
                                nc.gpsimd.collective_compute(
                                    "AllGather", Alu.bypass,
                                    ins=[emb_la[:]], outs=[emb_ga[:]],
                                    replica_groups=[list(range(N_CORES))])
                        else:
                            nc.sync.dma_start(
                                out=emb_lb[n0t - 5 * P:n0t - 5 * P + nn_t, :],
                                in_=ebf[:nn_t, :])
                    else:
                        nc.vector.tensor_scalar_add(
                            ebuf[:, t * HID:(t + 1) * HID], eadd[:], -1.0)
                if layer == 2:
                    for t in range(T):
                        nn_t = min(P, nloc - t * P)
                        nc.tensor.matmul(gps[0:1, :], lhsT=ones_col[:nn_t, :],
                                         rhs=ebuf[:nn_t, t * HID:(t + 1) * HID],
                                         start=(t == 0), stop=(t == T - 1))
                return gps

            # ---------------- main flow ----------------
            def zero_out_vec():
                z = kpool.tile([1, HID], dt.float32, tag="gout", name="z")
                nc.gpsimd.memset(z[:], 0.0)
                nc.sync.dma_start(out=out_vec[:], in_=z[:])

            def flow():
                project(xtv, w1s, K1, table1, 0, N_NODES, pre_t=True)
                if stages >= 2:
                    edges(table1, layer=1)
                if stages >= 3:
                    nc.gpsimd.collective_compute(
                        "AllGather", Alu.bypass,
                        ins=[emb_lb[:]], outs=[emb_gb[:]],
                        replica_groups=[list(range(N_CORES))])
                if stages >= 4:
                    # reload gather indices for layer 2 while projecting
                    nc.sync.dma_start(out=srcI[:], in_=idx_in["src16_2"][:])
                    nc.sync.dma_start(out=dstI[:], in_=idx_in["dst16_2"][:])
                    project(emb_ga, w2s, 1, table2, 0, NP2, pre_t=False)
                    project(emb_gb, w2s, 1, table2, NP2, NP2, pre_t=False)
                if stages == 5:
                    gps = edges(table2, layer=2)
                    gout = kpool.tile([1, HID], dt.float32, tag="gout")
                    nc.vector.tensor_copy(gout[:], gps[:])
                    nc.sync.dma_start(out=out_vec[:], in_=gout[:])
                else:
                    zero_out_vec()

            for _it in range(iters):
                flow()

    nc.compile()
    return nc


# ----------------------------------------------------------------------------
# top-level kernel
# ----------------------------------------------------------------------------

_CACHE = {}


def _run_device(in_maps, meta):
    from concourse.bass_utils import run_bass_kernel_spmd
    key = "prog"
    if key not in _CACHE:
        _CACHE[key] = build_program(meta)
    nc = _CACHE[key]
    res = run_bass_kernel_spmd(nc, in_maps, core_ids=list(range(N_CORES)))
    return res


def host_finish(partials, ln_g, ln_b, Wl1, bl1, Wl2, bl2, Wl3, bl3):
    g = partials.sum(axis=0) / np.float64(N_NODES)  # [HID]
    mu = g.mean()
    var = ((g - mu) ** 2).mean()
    gn = (g - mu) / np.sqrt(var + 1e-5) * ln_g + ln_b
    x = Wl1 @ gn + bl1
    x = np.maximum(x, 0.01 * x)
    x = Wl2 @ x + bl2
    x = np.maximum(x, 0.01 * x)
    x = Wl3 @ x + bl3
    return np.maximum(x, 0.0).astype(np.float32)


def kernel(node_features, edge_src, edge_dst, W1, a1, W2, a2,
           ln_g, ln_b, Wl1, bl1, Wl2, bl2, Wl3, bl3):
    node_features = np.asarray(node_features, dtype=np.float32)
    edge_src = np.asarray(edge_src, dtype=np.int32)
    edge_dst = np.asarray(edge_dst, dtype=np.int32)
    in_maps, meta = host_prep(node_features, edge_src, edge_dst,
                              np.asarray(W1, np.float32), np.asarray(a1, np.float32),
                              np.asarray(W2, np.float32), np.asarray(a2, np.float32))
    res = _run_device(in_maps, meta)
    partials = np.stack([res.results[c]["out_vec"][0] for c in range(N_CORES)])
    return host_finish(partials.astype(np.float64),
                       np.asarray(ln_g, np.float64), np.asarray(ln_b, np.float64),
                       np.asarray(Wl1, np.float64), np.asarray(bl1, np.float64),
                       np.asarray(Wl2, np.float64), np.asarray(bl2, np.float64),
                       np.asarray(Wl3, np.float64), np.asarray(bl3, np.float64))


# revision 17
# speedup vs baseline: 3.3711x; 3.3711x over previous
"""GAT (2-layer, 8-head) Trainium2 Bass kernel, SPMD over 8 NeuronCores.

Strategy (dst-node-parallel with replicated projection):
  - Host: sort edges by dst, give each core the edges whose dst falls in its
    1/8 node range. Precompute fused projection matrices Wcat = [W_heads | a_sj | a_si]
    (score halves pre-scaled x16 for fp8 range) so one matmul produces per-node
    head embeddings AND both attention score halves. Also precompute the
    per-chunk one-hot dst matrices and a pre-transposed copy of x.
  - Device, per core (single NEFF):
      Phase A: project ALL nodes (replicated) into an fp8 row table
               table[r(n)] = [h (1024 fp8) | sj x16 (8 fp8) | si x16 (8 fp8) | pad],
               row order r(n) permuted partition-major within 512-node blocks so
               the table store is one cheap SWDGE cast-DMA per block.
      Phase B: per owned 128-node tile: dma_gather rows by src (1280B) and
               score tails by dst (256B), leaky+exp (exp carries the 1/16
               descale), aggregate via onehot-matmul into PSUM (denominator =
               extra matmul columns), per-head normalize on ACT, head-mean+ELU.
      Phase C: AllGather emb1, SPLIT in two pipelined halves (tiles 0-4 ->
               AG-a issued mid-edge-phase, tiles 5-9 -> AG-b) with shards
               padded to 640 rows so layer-2 projection blocks stay 512-row
               aligned. AG-a overlaps the tail of the edge phase; AG-b
               overlaps layer-2 projection of the first half.
      Phase D/E: same for layer 2 -> per-core partial graph-sum [1, 128].
  - Host: sum partials, mean, LayerNorm + 3-layer MLP head -> [16].
"""

import numpy as np
import ml_dtypes

BF16 = ml_dtypes.bfloat16

# problem sizes (fixed for this problem)
N_NODES = 10000
N_EDGES = 160000
N_FEAT = 512
HEADS = 8
HID = 128
OUT = 16

N_CORES = 8
P = 128

SCORE_SCALE = 16.0  # sj/si stored x16 so fp8e4m3 keeps precision

NLOC = N_NODES // N_CORES   # 1250
HALF = 640                  # padded per-core shard rows per AllGather half
NP2 = N_CORES * HALF        # 5120 rows per gathered half (10 blocks of 512)


# ----------------------------------------------------------------------------
# host-side prep
# ----------------------------------------------------------------------------

def _wrap_idx(idx_i16: np.ndarray) -> np.ndarray:
    """[n] int16 -> [128, n//16] wrapped layout for dma_gather (16-partition
    wrap, replicated 8x down the partitions)."""
    n = idx_i16.shape[0]
    assert n % 16 == 0
    w = idx_i16.reshape(n // 16, 16).T  # [16, n//16]
    return np.tile(w, (8, 1))  # [128, n//16]


def _perm(n: np.ndarray) -> np.ndarray:
    """Row remap: node n -> table row. Within each 512-node block, rows are
    stored partition-major ((n%512)%128 outer, (n%512)//128 inner) so the
    projection's [P, SB, cols] SBUF tile lands contiguously per partition."""
    blk = n // 512
    off = n % 512
    return blk * 512 + (off % 128) * 4 + off // 128


def _pad2(n: np.ndarray) -> np.ndarray:
    """Node id -> padded layer-2 row id (per-core shards padded to 2x HALF,
    first halves concatenated then second halves)."""
    c = n // NLOC
    off = n % NLOC
    return np.where(off < HALF, c * HALF + off,
                    NP2 + c * HALF + (off - HALF))


def host_prep(node_features, edge_src, edge_dst, W1, a1, W2, a2):
    """Returns (in_maps, meta). meta carries static structure for the builder."""
    nloc = NLOC
    T = -(-nloc // P)  # owned tiles per core (10)

    order = np.argsort(edge_dst, kind="stable")
    src_s = edge_src[order].astype(np.int64)
    dst_s = edge_dst[order].astype(np.int64)

    # common chunk count per tile position (same across cores; program is SPMD)
    tile_cnt = np.zeros((N_CORES, T), dtype=np.int64)
    for c in range(N_CORES):
        base = c * nloc
        for t in range(T):
            n0 = base + t * P
            n1 = min(base + (t + 1) * P, (c + 1) * nloc)
            lo = np.searchsorted(dst_s, n0, side="left")
            hi = np.searchsorted(dst_s, n1, side="left")
            tile_cnt[c, t] = hi - lo
    C_t = [int(-(-tile_cnt[:, t].max() // P)) for t in range(T)]  # chunks per tile pos

    SLOTS = [c * P for c in C_t]
    tot_chunks = sum(C_t)
    tot_slots = tot_chunks * P

    in_maps = []
    xtv = np.ascontiguousarray(node_features.astype(BF16).T)  # [512, 10000]

    # fused projection matrices; score halves pre-scaled for fp8 storage
    def wcat(W, a, K):
        w = np.transpose(W, (2, 0, 1)).reshape(K, HEADS * HID)  # [K, H*D]
        si = np.einsum("hdf,hd->fh", W, a[:, :HID]) * SCORE_SCALE  # [K, H] dst-side
        sj = np.einsum("hdf,hd->fh", W, a[:, HID:]) * SCORE_SCALE  # [K, H] src-side
        return np.concatenate([w, sj, si], axis=1).astype(BF16)  # [K, H*D+16]

    w1c = np.ascontiguousarray(wcat(W1, a1, N_FEAT))
    w2c = np.ascontiguousarray(wcat(W2, a2, HID))

    def widx(ids, t_ranges):
        return np.concatenate(
            [_wrap_idx(ids[a:b].astype(np.int16)) for (a, b) in t_ranges], axis=1)

    for c in range(N_CORES):
        base = c * nloc
        src_pad = np.zeros(tot_slots, dtype=np.int64)
        dst_pad = np.zeros(tot_slots, dtype=np.int64)
        dstl_pad = np.full(tot_slots, 30000.0, dtype=np.float32)
        off = 0
        for t in range(T):
            n0 = base + t * P
            n1 = min(base + (t + 1) * P, (c + 1) * nloc)
            lo = np.searchsorted(dst_s, n0, side="left")
            hi = np.searchsorted(dst_s, n1, side="left")
            k = hi - lo
            sl = SLOTS[t]
            src_pad[off:off + k] = src_s[lo:hi]
            dst_pad[off:off + k] = dst_s[lo:hi]
            dstl_pad[off:off + k] = (dst_s[lo:hi] - n0).astype(np.float32)
            # padding: src/dst index 0 (valid gather), dstl 30000 (never matches)
            off += sl
        assert off == tot_slots

        t_ranges = [(sum(SLOTS[:t]), sum(SLOTS[:t + 1])) for t in range(T)]
        # gather rows: perm over raw node ids (same table layout both layers)
        src16 = widx(_perm(src_pad), t_ranges)
        dst16 = widx(_perm(dst_pad), t_ranges)

        # one-hot dst matrices, built on host: ohm[p, g*128+d] = (dstl[g*128+p]==d)
        dstl_cols = dstl_pad.reshape(tot_chunks, P)  # [chunk, edge-in-chunk]
        ohm = (dstl_cols[:, :, None] == np.arange(P)[None, None, :])  # [g, p, d]
        ohm = np.ascontiguousarray(
            ohm.transpose(1, 0, 2).reshape(P, tot_chunks * P).astype(BF16))

        in_maps.append({
            "xtv": xtv,
            "w1c": w1c,
            "w2c": w2c,
            "src16": np.ascontiguousarray(src16),
            "dst16": np.ascontiguousarray(dst16),
            "ohm": ohm,
        })

    meta = {
        "T": T, "C_t": C_t, "nloc": nloc,
        "tot_chunks": tot_chunks,
    }
    return in_maps, meta


# ----------------------------------------------------------------------------
# device program
# ----------------------------------------------------------------------------

HD = HEADS * HID          # 1024
ROW = 1280                # fp8 bytes per table row (1040 used, %256)
PCOLS = HD + 16           # projection output cols: [h (1024) | sj 8 | si 8]
TAIL = 256                # score-tail gather bytes (row bytes 1024:1280)


def build_program(meta, debug=False, stages=5, iters=1, ablate=""):
    import concourse.bacc as bacc
    import concourse.mybir as mybir
    import concourse.tile as tile
    from concourse.library_config import mlp

    dt = mybir.dt
    Alu = mybir.AluOpType
    Act = mybir.ActivationFunctionType

    T = meta["T"]
    C_t = meta["C_t"]
    nloc = meta["nloc"]
    tot_chunks = meta["tot_chunks"]

    K1 = N_FEAT // P       # 4 contraction chunks layer 1
    RB = 512               # rows per projection block

    nc = bacc.Bacc("TRN2", num_devices=N_CORES, num_swdge_queues=2,
                   dynamic_dma_scratch_size=49152)

    xtv = nc.dram_tensor("xtv", [N_FEAT, N_NODES], dt.bfloat16, kind="ExternalInput")
    w1c = nc.dram_tensor("w1c", [N_FEAT, PCOLS], dt.bfloat16, kind="ExternalInput")
    w2c = nc.dram_tensor("w2c", [HID, PCOLS], dt.bfloat16, kind="ExternalInput")
    src16 = nc.dram_tensor("src16", [P, tot_chunks * 8], dt.int16, kind="ExternalInput")
    dst16 = nc.dram_tensor("dst16", [P, tot_chunks * 8], dt.int16, kind="ExternalInput")
    ohm = nc.dram_tensor("ohm", [P, tot_chunks * P], dt.bfloat16, kind="ExternalInput")

    out_vec = nc.dram_tensor("out_vec", [1, HID], dt.float32, kind="ExternalOutput")

    NPAD1 = (-(-N_NODES // RB)) * RB  # 10240
    table1 = nc.dram_tensor("table1", [NPAD1, ROW], dt.float8e4)
    table2 = nc.dram_tensor("table2", [NPAD1, ROW], dt.float8e4)
    emb_loc = nc.dram_tensor("emb_loc", [nloc, HID], dt.bfloat16)
    emb_all = nc.dram_tensor("emb_all", [N_NODES, HID], dt.bfloat16,
                             addr_space="Shared")

    with tile.TileContext(nc) as tc:
        with (
            tc.tile_pool(name="const", bufs=1) as cpool,
            tc.tile_pool(name="xtp", bufs=8) as tpool,
            tc.tile_pool(name="work", bufs=3) as wpool,
            tc.tile_pool(name="rowp", bufs=2) as rpool,
            tc.tile_pool(name="chunk", bufs=3) as kpool,
            tc.tile_pool(name="psum", bufs=2, space="PSUM") as pspool,
            tc.tile_pool(name="psg", bufs=1, space="PSUM") as psg,
        ):
            nc.gpsimd.load_library(mlp)

            # ---- constants ----
            ones_col = cpool.tile([P, 1], dt.bfloat16)
            nc.gpsimd.memset(ones_col[:], 1.0)

            w1s = cpool.tile([P, K1 * PCOLS], dt.bfloat16)
            nc.sync.dma_start(
                out=w1s[:].rearrange("p (k c) -> p k c", k=K1),
                in_=w1c[:].rearrange("(k p) c -> p k c", p=P))
            w2s = cpool.tile([P, PCOLS], dt.bfloat16)
            nc.sync.dma_start(out=w2s[:], in_=w2c[:])

            srcI = cpool.tile([P, tot_chunks * 8], dt.int16)
            dstI = cpool.tile([P, tot_chunks * 8], dt.int16)
            nc.sync.dma_start(out=srcI[:], in_=src16[:])
            nc.sync.dma_start(out=dstI[:], in_=dst16[:])

            # ---------------- projection phase ----------------
            def project(src_dram, wtile, K, table, row0, nrows, pre_t):
                """table[row0 + r(n)] = [h | sj | si] (fp8, x16 scores).
                pre_t: src_dram is already transposed ([K*P, nrows])."""
                nblk = -(-nrows // RB)
                for b in range(nblk):
                    r0 = b * RB
                    rn = min(RB, nrows - r0)
                    xT = []
                    for k in range(K):
                        t_ = tpool.tile([P, RB], dt.bfloat16, tag="xT")
                        eng = nc.sync if k % 2 == 0 else nc.scalar
                        if pre_t:
                            eng.dma_start(
                                out=t_[:, :rn],
                                in_=src_dram[k * P:(k + 1) * P, r0:r0 + rn])
                        else:
                            eng.dma_start_transpose(
                                t_[:, :rn], src_dram[r0:r0 + rn, k * P:(k + 1) * P])
                        xT.append(t_)
                    SB = RB // P
                    row = rpool.tile([P, SB, ROW], dt.bfloat16, tag="row")
                    for s in range(SB):
                        n0 = r0 + s * P
                        if n0 >= nrows:
                            break
                        nn = min(P, nrows - n0)
                        ps = pspool.tile([P, PCOLS], dt.float32, tag="ps")
                        for k in range(K):
                            lhsT = xT[k][:, s * P:s * P + nn]
                            rhs = wtile[:, k * PCOLS:(k + 1) * PCOLS] if K > 1 \
                                else wtile[:]
                            st, sp = (k == 0), (k == K - 1)
                            nc.tensor.matmul(ps[:nn, 0:512], lhsT=lhsT,
                                             rhs=rhs[:, 0:512], start=st, stop=sp)
                            nc.tensor.matmul(ps[:nn, 512:1024], lhsT=lhsT,
                                             rhs=rhs[:, 512:1024], start=st, stop=sp)
                            nc.tensor.matmul(ps[:nn, 1024:PCOLS], lhsT=lhsT,
                                             rhs=rhs[:, 1024:PCOLS], start=st, stop=sp)
                        # drain split across DVE and ACT
                        nc.vector.tensor_copy(row[:nn, s, 0:512], ps[:nn, 0:512])
                        nc.scalar.activation(row[:nn, s, 512:PCOLS],
                                             ps[:nn, 512:PCOLS], Act.Copy)
                    # one SWDGE cast-DMA (bf16->fp8): rows partition-major.
                    # Tail block writes junk rows into the table pad — never
                    # referenced by any gather index.
                    nc.gpsimd.dma_start(
                        out=table[row0 + r0:row0 + r0 + RB, :].rearrange(
                            "(p s) c -> p s c", p=P),
                        in_=row[:])

            # ---------------- edge phase ----------------
            def edges(table, layer):
                gps = psg.tile([1, HID], dt.float32, tag="gsum", name="gps") if layer == 2 else None
                ebuf = cpool.tile([P, T * HID], dt.bfloat16, name="ebuf") if layer == 2 else None
                for t in range(T):
                    C = C_t[t]
                    ioff = sum(C_t[:t])
                    n0t = t * P
                    nn_t = min(P, nloc - n0t)
                    G = wpool.tile([P, C, ROW], dt.float8e4, tag="G")
                    SIG = wpool.tile([P, C, TAIL], dt.float8e4, tag="SIG")
                    ohT = wpool.tile([P, C, P], dt.bfloat16, tag="ohT")
                    nc.scalar.dma_start(
                        out=ohT[:], in_=ohm[:, ioff * P:(ioff + C) * P].rearrange(
                            "p (c d) -> p c d", c=C))
                    Ch = C // 2
                    nc.gpsimd.dma_gather(
                        G[:, 0:Ch, :], table[:], srcI[:, ioff * 8:(ioff + Ch) * 8],
                        Ch * P, Ch * P, ROW, single_packet=False, queue_num=0)
                    nc.gpsimd.dma_gather(
                        G[:, Ch:C, :], table[:],
                        srcI[:, (ioff + Ch) * 8:(ioff + C) * 8],
                        (C - Ch) * P, (C - Ch) * P, ROW, single_packet=False,
                        queue_num=1)
                    nc.gpsimd.dma_gather(
                        SIG[:, 0:Ch, :], table[:, 1024:ROW],
                        dstI[:, ioff * 8:(ioff + Ch) * 8],
                        Ch * P, Ch * P, TAIL, elem_step=ROW,
                        single_packet=False, queue_num=1)
                    nc.gpsimd.dma_gather(
                        SIG[:, Ch:C, :], table[:, 1024:ROW],
                        dstI[:, (ioff + Ch) * 8:(ioff + C) * 8],
                        (C - Ch) * P, (C - Ch) * P, TAIL, elem_step=ROW,
                        single_packet=False, queue_num=0)
                    ps = pspool.tile([P, PCOLS], dt.float32, tag="ps")
                    # batched attention logits for the whole tile (x16 scale)
                    LG = wpool.tile([P, C * 8], dt.float32, tag="LG")
                    nc.vector.tensor_tensor(
                        out=LG[:].rearrange("p (c e) -> p c e", c=C),
                        in0=SIG[:, :, 8:16], in1=G[:, :, 1024:1032], op=Alu.add)
                    # leaky_relu on DVE (still x16), exp on ACT with scale 1/16
                    LR = wpool.tile([P, C * 8], dt.float32, tag="LR")
                    nc.vector.tensor_scalar_mul(LR[:], LG[:], 0.01)
                    MX = wpool.tile([P, C * 8], dt.float32, tag="MX")
                    nc.vector.tensor_tensor(out=MX[:], in0=LG[:], in1=LR[:],
                                            op=Alu.max)
                    EX = wpool.tile([P, C * 8], dt.float32, tag="EX")
                    nc.scalar.activation(EX[:], MX[:], Act.Exp,
                                         scale=1.0 / SCORE_SCALE)
                    EXbf = wpool.tile([P, C, 8], dt.bfloat16, tag="EXbf")
                    nc.vector.tensor_copy(
                        EXbf[:], EX[:].rearrange("p (c e) -> p c e", c=C))
                    for c in range(C):
                        msg = kpool.tile([P, HD], dt.bfloat16, tag="msg")
                        for h in range(HEADS):
                            if h in (3, 7):
                                nc.scalar.activation(
                                    msg[:, h * HID:(h + 1) * HID],
                                    G[:, c, h * HID:(h + 1) * HID],
                                    Act.Copy,
                                    scale=EX[:, c * 8 + h:c * 8 + h + 1])
                            else:
                                nc.vector.tensor_scalar_mul(
                                    msg[:, h * HID:(h + 1) * HID],
                                    G[:, c, h * HID:(h + 1) * HID],
                                    EX[:, c * 8 + h:c * 8 + h + 1])
                        lhs = ohT[:, c, :]
                        st, sp = (c == 0), (c == C - 1)
                        nc.tensor.matmul(ps[:, 0:512], lhsT=lhs,
                                         rhs=msg[:, 0:512], start=st, stop=sp)
                        nc.tensor.matmul(ps[:, 512:1024], lhsT=lhs,
                                         rhs=msg[:, 512:1024], start=st, stop=sp)
                        nc.tensor.matmul(ps[:, 1024:1032], lhsT=lhs,
                                         rhs=EXbf[:, c, :], start=st, stop=sp)
                    # ---- postprocess tile ----
                    den = kpool.tile([P, 8], dt.float32, tag="den")
                    nc.vector.tensor_scalar(out=den[:], in0=ps[:, 1024:1032],
                                            scalar1=float(HEADS), scalar2=1e-30,
                                            op0=Alu.mult, op1=Alu.max)
                    rec = kpool.tile([P, 8], dt.float32, tag="rec")
                    nc.vector.reciprocal(rec[:], den[:])
                    # per-head normalize on ACT (PSUM src), tree-add on DVE
                    hn = kpool.tile([P, HEADS, HID], dt.bfloat16, tag="hn")
                    for h in range(HEADS):
                        nc.scalar.activation(hn[:, h, :],
                                             ps[:, h * HID:(h + 1) * HID],
                                             Act.Copy, scale=rec[:, h:h + 1])
                    s4a = kpool.tile([P, 4, HID], dt.bfloat16, tag="s4a")
                    nc.vector.tensor_tensor(out=s4a[:], in0=hn[:, 0:4, :],
                                            in1=hn[:, 4:8, :], op=Alu.add)
                    s2a = kpool.tile([P, 2, HID], dt.bfloat16, tag="s2a")
                    nc.vector.tensor_tensor(out=s2a[:], in0=s4a[:, 0:2, :],
                                            in1=s4a[:, 2:4, :], op=Alu.add)
                    S = kpool.tile([P, HID], dt.float32, tag="S")
                    nc.vector.tensor_tensor(out=S[:], in0=s2a[:, 0, :],
                                            in1=s2a[:, 1, :], op=Alu.add)
                    # elu(S) = exp(min(S,0)) - 1 + max(S,0)
                    neg = kpool.tile([P, HID], dt.float32, tag="neg")
                    nc.vector.tensor_scalar_min(neg[:], S[:], 0.0)
                    en = kpool.tile([P, HID], dt.float32, tag="en")
                    nc.scalar.activation(en[:], neg[:], Act.Exp)
                    pos = kpool.tile([P, HID], dt.float32, tag="pos")
                    nc.vector.tensor_scalar_max(pos[:], S[:], 0.0)
                    eadd = kpool.tile([P, HID], dt.float32, tag="eadd")
                    nc.vector.tensor_tensor(out=eadd[:], in0=en[:], in1=pos[:],
                                            op=Alu.add)
                    if layer == 1:
                        ebf = kpool.tile([P, HID], dt.bfloat16, tag="ebf")
                        nc.vector.tensor_scalar_add(ebf[:], eadd[:], -1.0)
                        nc.sync.dma_start(out=emb_loc[n0t:n0t + nn_t, :],
                                          in_=ebf[:nn_t, :])
                    else:
                        nc.vector.tensor_scalar_add(
                            ebuf[:, t * HID:(t + 1) * HID], eadd[:], -1.0)
                if layer == 2:
                    for t in range(T):
                        nn_t = min(P, nloc - t * P)
                        nc.tensor.matmul(gps[0:1, :], lhsT=ones_col[:nn_t, :],
                                         rhs=ebuf[:nn_t, t * HID:(t + 1) * HID],
                                         start=(t == 0), stop=(t == T - 1))
                return gps

            # ---------------- main flow ----------------
            def zero_out_vec():
                z = kpool.tile([1, HID], dt.float32, tag="gout", name="z")
                nc.gpsimd.memset(z[:], 0.0)
                nc.sync.dma_start(out=out_vec[:], in_=z[:])

            def flow():
                project(xtv, w1s, K1, table1, 0, N_NODES, pre_t=True)
                if stages >= 2:
                    edges(table1, layer=1)
                if stages >= 3:
                    nc.gpsimd.collective_compute(
                        "AllGather", Alu.bypass,
                        ins=[emb_loc[:]], outs=[emb_all[:]],
                        replica_groups=[list(range(N_CORES))])
                if stages >= 4:
                    project(emb_all, w2s, 1, table2, 0, N_NODES, pre_t=False)
                if stages == 5:
                    gps = edges(table2, layer=2)
                    gout = kpool.tile([1, HID], dt.float32, tag="gout")
                    nc.vector.tensor_copy(gout[:], gps[:])
                    nc.sync.dma_start(out=out_vec[:], in_=gout[:])
                else:
                    zero_out_vec()

            for _it in range(iters):
                flow()

    nc.compile()
    return nc


# ----------------------------------------------------------------------------
# top-level kernel
# ----------------------------------------------------------------------------

_CACHE = {}


def _run_device(in_maps, meta):
    from concourse.bass_utils import run_bass_kernel_spmd
    key = "prog"
    if key not in _CACHE:
        _CACHE[key] = build_program(meta)
    nc = _CACHE[key]
    res = run_bass_kernel_spmd(nc, in_maps, core_ids=list(range(N_CORES)))
    return res


def host_finish(partials, ln_g, ln_b, Wl1, bl1, Wl2, bl2, Wl3, bl3):
    g = partials.sum(axis=0) / np.float64(N_NODES)  # [HID]
    mu = g.mean()
    var = ((g - mu) ** 2).mean()
    gn = (g - mu) / np.sqrt(var + 1e-5) * ln_g + ln_b
    x = Wl1 @ gn + bl1
    x = np.maximum(x, 0.01 * x)
    x = Wl2 @ x + bl2
    x = np.maximum(x, 0.01 * x)
    x = Wl3 @ x + bl3
    return np.maximum(x, 0.0).astype(np.float32)


def kernel(node_features, edge_src, edge_dst, W1, a1, W2, a2,
           ln_g, ln_b, Wl1, bl1, Wl2, bl2, Wl3, bl3):
    node_features = np.asarray(node_features, dtype=np.float32)
    edge_src = np.asarray(edge_src, dtype=np.int32)
    edge_dst = np.asarray(edge_dst, dtype=np.int32)
    in_maps, meta = host_prep(node_features, edge_src, edge_dst,
                              np.asarray(W1, np.float32), np.asarray(a1, np.float32),
                              np.asarray(W2, np.float32), np.asarray(a2, np.float32))
    res = _run_device(in_maps, meta)
    partials = np.stack([res.results[c]["out_vec"][0] for c in range(N_CORES)])
    return host_finish(partials.astype(np.float64),
                       np.asarray(ln_g, np.float64), np.asarray(ln_b, np.float64),
                       np.asarray(Wl1, np.float64), np.asarray(bl1, np.float64),
                       np.asarray(Wl2, np.float64), np.asarray(bl2, np.float64),
                       np.asarray(Wl3, np.float64), np.asarray(bl3, np.float64))


# revision 18
# speedup vs baseline: 5.0148x; 1.4876x over previous
"""GAT (2-layer, 8-head) Trainium2 Bass kernel, SPMD over 8 NeuronCores.

Strategy (dst-node-parallel with replicated projection):
  - Host: sort edges by dst, give each core the edges whose dst falls in its
    1/8 node range. Precompute fused projection matrices Wcat = [W_heads | a_sj | a_si]
    (score halves pre-scaled x16 for fp8 range) so one matmul produces per-node
    head embeddings AND both attention score halves. Also precompute the
    per-chunk one-hot dst matrices (static) so the device never builds them.
  - Device, per core (single NEFF):
      Phase A: project ALL nodes (replicated) into an fp8 row table
               table[r(n)] = [h (1024 fp8) | sj x16 (8 fp8) | si x16 (8 fp8) | pad],
               row order r(n) permuted partition-major within 512-node blocks so
               the table store is one cheap SWDGE cast-DMA per block.
      Phase B: for each owned 128-node tile: dma_gather rows by src (1280B) and
               score tails by dst (256B), leaky+exp via ACT (input scale 1/16),
               aggregate via onehot-matmul into PSUM (denominator = extra
               matmul columns), per-head normalize on ACT, head-mean + ELU.
      Phase C: AllGather emb1 (bf16, 2.5MB).
      Phase D/E: same for layer 2 -> per-core partial graph-sum [1, 128].
  - Host: sum partials, mean, LayerNorm + 3-layer MLP head -> [16].
"""

import numpy as np
import ml_dtypes

BF16 = ml_dtypes.bfloat16

# problem sizes (fixed for this problem)
N_NODES = 10000
N_EDGES = 160000
N_FEAT = 512
HEADS = 8
HID = 128
OUT = 16

N_CORES = 8
P = 128

SCORE_SCALE = 16.0  # sj/si stored x16 so fp8e4m3 keeps precision


# ----------------------------------------------------------------------------
# host-side prep
# ----------------------------------------------------------------------------

def _wrap_idx(idx_i16: np.ndarray) -> np.ndarray:
    """[n] int16 -> [128, n//16] wrapped layout for dma_gather (16-partition
    wrap, replicated 8x down the partitions)."""
    n = idx_i16.shape[0]
    assert n % 16 == 0
    w = idx_i16.reshape(n // 16, 16).T  # [16, n//16]
    return np.tile(w, (8, 1))  # [128, n//16]


def _perm(n: np.ndarray) -> np.ndarray:
    """Row remap: node n -> table row. Within each 512-node block, rows are
    stored partition-major ((n%512)%128 outer, (n%512)//128 inner) so the
    projection's [P, SB, cols] SBUF tile lands contiguously per partition."""
    blk = n // 512
    off = n % 512
    return blk * 512 + (off % 128) * 4 + off // 128


def host_prep(node_features, edge_src, edge_dst, W1, a1, W2, a2):
    """Returns (in_maps, meta). meta carries static structure for the builder."""
    nloc = N_NODES // N_CORES  # 1250
    T = -(-nloc // P)  # owned tiles per core (10)

    order = np.argsort(edge_dst, kind="stable")
    src_s = edge_src[order].astype(np.int64)
    dst_s = edge_dst[order].astype(np.int64)

    # common chunk count per tile position (same across cores; program is SPMD)
    tile_cnt = np.zeros((N_CORES, T), dtype=np.int64)
    for c in range(N_CORES):
        base = c * nloc
        for t in range(T):
            n0 = base + t * P
            n1 = min(base + (t + 1) * P, (c + 1) * nloc)
            lo = np.searchsorted(dst_s, n0, side="left")
            hi = np.searchsorted(dst_s, n1, side="left")
            tile_cnt[c, t] = hi - lo
    C_t = [int(-(-tile_cnt[:, t].max() // P)) for t in range(T)]  # chunks per tile pos

    SLOTS = [c * P for c in C_t]
    tot_chunks = sum(C_t)
    tot_slots = tot_chunks * P

    in_maps = []
    xbf = np.ascontiguousarray(node_features.astype(BF16))

    # fused projection matrices; score halves pre-scaled for fp8 storage
    def wcat(W, a, K):
        w = np.transpose(W, (2, 0, 1)).reshape(K, HEADS * HID)  # [K, H*D]
        si = np.einsum("hdf,hd->fh", W, a[:, :HID]) * SCORE_SCALE  # [K, H] dst-side
        sj = np.einsum("hdf,hd->fh", W, a[:, HID:]) * SCORE_SCALE  # [K, H] src-side
        return np.concatenate([w, sj, si], axis=1).astype(BF16)  # [K, H*D+16]

    w1c = np.ascontiguousarray(wcat(W1, a1, N_FEAT))
    w2c = np.ascontiguousarray(wcat(W2, a2, HID))

    for c in range(N_CORES):
        base = c * nloc
        src_pad = np.zeros(tot_slots, dtype=np.int64)
        dst_pad = np.zeros(tot_slots, dtype=np.int64)
        dstl_pad = np.full(tot_slots, 30000.0, dtype=np.float32)
        off = 0
        for t in range(T):
            n0 = base + t * P
            n1 = min(base + (t + 1) * P, (c + 1) * nloc)
            lo = np.searchsorted(dst_s, n0, side="left")
            hi = np.searchsorted(dst_s, n1, side="left")
            k = hi - lo
            sl = SLOTS[t]
            src_pad[off:off + k] = src_s[lo:hi]
            dst_pad[off:off + k] = dst_s[lo:hi]
            dstl_pad[off:off + k] = (dst_s[lo:hi] - n0).astype(np.float32)
            # padding: src/dst index 0 (valid gather), dstl 30000 (never matches)
            off += sl
        assert off == tot_slots

        # wrapped int16 gather indices (row-permuted), per tile concatenated
        src_r = _perm(src_pad)
        dst_r = _perm(dst_pad)
        src16 = np.concatenate(
            [_wrap_idx(src_r[sum(SLOTS[:t]):sum(SLOTS[:t + 1])].astype(np.int16))
             for t in range(T)], axis=1)
        dst16 = np.concatenate(
            [_wrap_idx(dst_r[sum(SLOTS[:t]):sum(SLOTS[:t + 1])].astype(np.int16))
             for t in range(T)], axis=1)
        # one-hot dst matrices, built on host: ohm[p, g*128+d] = (dstl[g*128+p]==d)
        dstl_cols = dstl_pad.reshape(tot_chunks, P)  # [chunk, edge-in-chunk]
        ohm = (dstl_cols[:, :, None] == np.arange(P)[None, None, :])  # [g, p, d]
        ohm = np.ascontiguousarray(
            ohm.transpose(1, 0, 2).reshape(P, tot_chunks * P).astype(BF16))

        in_maps.append({
            "xbf": xbf,
            "w1c": w1c,
            "w2c": w2c,
            "src16": np.ascontiguousarray(src16),
            "dst16": np.ascontiguousarray(dst16),
            "ohm": ohm,
        })

    meta = {
        "T": T, "C_t": C_t, "nloc": nloc,
        "tot_chunks": tot_chunks,
    }
    return in_maps, meta


# ----------------------------------------------------------------------------
# device program
# ----------------------------------------------------------------------------

HD = HEADS * HID          # 1024
ROW = 1280                # fp8 bytes per table row (1040 used, %256)
PCOLS = HD + 16           # projection output cols: [h (1024) | sj 8 | si 8]
TAIL = 256                # score-tail gather bytes (row bytes 1024:1280)


def build_program(meta, debug=False, stages=5, iters=1, ablate=""):
    import concourse.bacc as bacc
    import concourse.mybir as mybir
    import concourse.tile as tile
    from concourse.library_config import mlp

    dt = mybir.dt
    Alu = mybir.AluOpType
    Act = mybir.ActivationFunctionType

    T = meta["T"]
    C_t = meta["C_t"]
    nloc = meta["nloc"]
    tot_chunks = meta["tot_chunks"]

    K1 = N_FEAT // P       # 4 contraction chunks layer 1
    RB = 512               # rows per projection block

    nc = bacc.Bacc("TRN2", num_devices=N_CORES, num_swdge_queues=2,
                   dynamic_dma_scratch_size=49152)

    xbf = nc.dram_tensor("xbf", [N_NODES, N_FEAT], dt.bfloat16, kind="ExternalInput")
    w1c = nc.dram_tensor("w1c", [N_FEAT, PCOLS], dt.bfloat16, kind="ExternalInput")
    w2c = nc.dram_tensor("w2c", [HID, PCOLS], dt.bfloat16, kind="ExternalInput")
    src16 = nc.dram_tensor("src16", [P, tot_chunks * 8], dt.int16, kind="ExternalInput")
    dst16 = nc.dram_tensor("dst16", [P, tot_chunks * 8], dt.int16, kind="ExternalInput")
    ohm = nc.dram_tensor("ohm", [P, tot_chunks * P], dt.bfloat16, kind="ExternalInput")

    out_vec = nc.dram_tensor("out_vec", [1, HID], dt.float32, kind="ExternalOutput")

    NPAD = (-(-N_NODES // RB)) * RB  # tables padded to whole blocks (10240)
    table1 = nc.dram_tensor("table1", [NPAD, ROW], dt.float8e4)
    table2 = nc.dram_tensor("table2", [NPAD, ROW], dt.float8e4)
    emb_loc = nc.dram_tensor("emb_loc", [nloc, HID], dt.bfloat16)
    emb_all = nc.dram_tensor("emb_all", [N_NODES, HID], dt.bfloat16,
                             addr_space="Shared")

    with tile.TileContext(nc) as tc:
        with (
            tc.tile_pool(name="const", bufs=1) as cpool,
            tc.tile_pool(name="xtp", bufs=8) as tpool,
            tc.tile_pool(name="work", bufs=2) as wpool,
            tc.tile_pool(name="chunk", bufs=3) as kpool,
            tc.tile_pool(name="psum", bufs=2, space="PSUM") as pspool,
            tc.tile_pool(name="psg", bufs=1, space="PSUM") as psg,
        ):
            nc.gpsimd.load_library(mlp)

            # ---- constants ----
            ones_col = cpool.tile([P, 1], dt.bfloat16)
            nc.gpsimd.memset(ones_col[:], 1.0)

            w1s = cpool.tile([P, K1 * PCOLS], dt.bfloat16)
            nc.sync.dma_start(
                out=w1s[:].rearrange("p (k c) -> p k c", k=K1),
                in_=w1c[:].rearrange("(k p) c -> p k c", p=P))
            w2s = cpool.tile([P, PCOLS], dt.bfloat16)
            nc.sync.dma_start(out=w2s[:], in_=w2c[:])

            srcI = cpool.tile([P, tot_chunks * 8], dt.int16)
            nc.sync.dma_start(out=srcI[:], in_=src16[:])
            dstI = cpool.tile([P, tot_chunks * 8], dt.int16)
            nc.sync.dma_start(out=dstI[:], in_=dst16[:])
            ohS = cpool.tile([P, tot_chunks * P], dt.bfloat16)
            nc.sync.dma_start(out=ohS[:], in_=ohm[:])

            # ---------------- projection phase ----------------
            def project(src_dram, wtile, K, table):
                """table[r(n)] = [h | sj | si] (fp8, x16 scores) for all nodes."""
                nblk = -(-N_NODES // RB)
                for b in range(nblk):
                    r0 = b * RB
                    rn = min(RB, N_NODES - r0)
                    xT = []
                    for k in range(K):
                        t_ = tpool.tile([P, RB], dt.bfloat16, tag="xT")
                        eng = nc.sync if k % 2 == 0 else nc.scalar
                        eng.dma_start_transpose(
                            t_[:, :rn], src_dram[r0:r0 + rn, k * P:(k + 1) * P])
                        xT.append(t_)
                    SB = RB // P
                    row = wpool.tile([P, SB, ROW], dt.bfloat16, tag="row")
                    for s in range(SB):
                        n0 = r0 + s * P
                        if n0 >= N_NODES:
                            break
                        nn = min(P, N_NODES - n0)
                        ps = pspool.tile([P, PCOLS], dt.float32, tag="ps")
                        for k in range(K):
                            lhsT = xT[k][:, s * P:s * P + nn]
                            rhs = wtile[:, k * PCOLS:(k + 1) * PCOLS] if K > 1 \
                                else wtile[:]
                            st, sp = (k == 0), (k == K - 1)
                            nc.tensor.matmul(ps[:nn, 0:512], lhsT=lhsT,
                                             rhs=rhs[:, 0:512], start=st, stop=sp)
                            nc.tensor.matmul(ps[:nn, 512:1024], lhsT=lhsT,
                                             rhs=rhs[:, 512:1024], start=st, stop=sp)
                            nc.tensor.matmul(ps[:nn, 1024:PCOLS], lhsT=lhsT,
                                             rhs=rhs[:, 1024:PCOLS], start=st, stop=sp)
                        # drain split across DVE and ACT
                        nc.vector.tensor_copy(row[:nn, s, 0:512], ps[:nn, 0:512])
                        nc.scalar.activation(row[:nn, s, 512:PCOLS],
                                             ps[:nn, 512:PCOLS], Act.Copy)
                    # one SWDGE cast-DMA (bf16->fp8): rows partition-major.
                    # Tail block writes junk rows into the table pad — never
                    # referenced by any gather index.
                    nc.gpsimd.dma_start(
                        out=table[r0:r0 + RB, :].rearrange(
                            "(p s) c -> p s c", p=P),
                        in_=row[:])

            # ---------------- edge phase ----------------
            def edges(table, layer):
                gps = psg.tile([1, HID], dt.float32, tag="gsum", name="gps") if layer == 2 else None
                ebuf = cpool.tile([P, T * HID], dt.bfloat16, name="ebuf") if layer == 2 else None
                for t in range(T):
                    C = C_t[t]
                    ioff = sum(C_t[:t])
                    n0t = t * P
                    nn_t = min(P, nloc - n0t)
                    G = wpool.tile([P, C, ROW], dt.float8e4, tag="G")
                    SIG = wpool.tile([P, C, TAIL], dt.float8e4, tag="SIG")
                    Ch = C // 2
                    nc.gpsimd.dma_gather(
                        G[:, 0:Ch, :], table[:], srcI[:, ioff * 8:(ioff + Ch) * 8],
                        Ch * P, Ch * P, ROW, single_packet=False, queue_num=0)
                    nc.gpsimd.dma_gather(
                        G[:, Ch:C, :], table[:],
                        srcI[:, (ioff + Ch) * 8:(ioff + C) * 8],
                        (C - Ch) * P, (C - Ch) * P, ROW, single_packet=False,
                        queue_num=1)
                    nc.gpsimd.dma_gather(
                        SIG[:, 0:Ch, :], table[:, 1024:ROW],
                        dstI[:, ioff * 8:(ioff + Ch) * 8],
                        Ch * P, Ch * P, TAIL, elem_step=ROW,
                        single_packet=False, queue_num=1)
                    nc.gpsimd.dma_gather(
                        SIG[:, Ch:C, :], table[:, 1024:ROW],
                        dstI[:, (ioff + Ch) * 8:(ioff + C) * 8],
                        (C - Ch) * P, (C - Ch) * P, TAIL, elem_step=ROW,
                        single_packet=False, queue_num=0)
                    ps = pspool.tile([P, PCOLS], dt.float32, tag="ps")
                    # batched attention logits for the whole tile (x16 scale)
                    LG = wpool.tile([P, C * 8], dt.float32, tag="LG")
                    nc.vector.tensor_tensor(
                        out=LG[:].rearrange("p (c e) -> p c e", c=C),
                        in0=SIG[:, :, 8:16], in1=G[:, :, 1024:1032], op=Alu.add)
                    # leaky_relu on DVE (still x16), exp on ACT with scale 1/16
                    LR = wpool.tile([P, C * 8], dt.float32, tag="LR")
                    nc.vector.tensor_scalar_mul(LR[:], LG[:], 0.01)
                    MX = wpool.tile([P, C * 8], dt.float32, tag="MX")
                    nc.vector.tensor_tensor(out=MX[:], in0=LG[:], in1=LR[:],
                                            op=Alu.max)
                    EX = wpool.tile([P, C * 8], dt.float32, tag="EX")
                    nc.scalar.activation(EX[:], MX[:], Act.Exp,
                                         scale=1.0 / SCORE_SCALE)
                    EXbf = wpool.tile([P, C, 8], dt.bfloat16, tag="EXbf")
                    nc.vector.tensor_copy(
                        EXbf[:], EX[:].rearrange("p (c e) -> p c e", c=C))
                    for c in range(C):
                        msg = kpool.tile([P, HD], dt.bfloat16, tag="msg")
                        for h in range(HEADS):
                            if h in (3, 7):
                                nc.scalar.activation(
                                    msg[:, h * HID:(h + 1) * HID],
                                    G[:, c, h * HID:(h + 1) * HID],
                                    Act.Copy,
                                    scale=EX[:, c * 8 + h:c * 8 + h + 1])
                            else:
                                nc.vector.tensor_scalar_mul(
                                    msg[:, h * HID:(h + 1) * HID],
                                    G[:, c, h * HID:(h + 1) * HID],
                                    EX[:, c * 8 + h:c * 8 + h + 1])
                        lhs = ohS[:, (ioff + c) * P:(ioff + c + 1) * P]
                        st, sp = (c == 0), (c == C - 1)
                        nc.tensor.matmul(ps[:, 0:512], lhsT=lhs,
                                         rhs=msg[:, 0:512], start=st, stop=sp)
                        nc.tensor.matmul(ps[:, 512:1024], lhsT=lhs,
                                         rhs=msg[:, 512:1024], start=st, stop=sp)
                        nc.tensor.matmul(ps[:, 1024:1032], lhsT=lhs,
                                         rhs=EXbf[:, c, :], start=st, stop=sp)
                    # ---- postprocess tile ----
                    den = kpool.tile([P, 8], dt.float32, tag="den")
                    nc.vector.tensor_scalar(out=den[:], in0=ps[:, 1024:1032],
                                            scalar1=float(HEADS), scalar2=1e-30,
                                            op0=Alu.mult, op1=Alu.max)
                    rec = kpool.tile([P, 8], dt.float32, tag="rec")
                    nc.vector.reciprocal(rec[:], den[:])
                    # per-head normalize on ACT (PSUM src), tree-add on DVE
                    hn = kpool.tile([P, HEADS, HID], dt.bfloat16, tag="hn")
                    for h in range(HEADS):
                        nc.scalar.activation(hn[:, h, :],
                                             ps[:, h * HID:(h + 1) * HID],
                                             Act.Copy, scale=rec[:, h:h + 1])
                    s4a = kpool.tile([P, 4, HID], dt.bfloat16, tag="s4a")
                    nc.vector.tensor_tensor(out=s4a[:], in0=hn[:, 0:4, :],
                                            in1=hn[:, 4:8, :], op=Alu.add)
                    s2a = kpool.tile([P, 2, HID], dt.bfloat16, tag="s2a")
                    nc.vector.tensor_tensor(out=s2a[:], in0=s4a[:, 0:2, :],
                                            in1=s4a[:, 2:4, :], op=Alu.add)
                    S = kpool.tile([P, HID], dt.float32, tag="S")
                    nc.vector.tensor_tensor(out=S[:], in0=s2a[:, 0, :],
                                            in1=s2a[:, 1, :], op=Alu.add)
                    # elu(S) = exp(min(S,0)) - 1 + max(S,0)
                    neg = kpool.tile([P, HID], dt.float32, tag="neg")
                    nc.vector.tensor_scalar_min(neg[:], S[:], 0.0)
                    en = kpool.tile([P, HID], dt.float32, tag="en")
                    nc.scalar.activation(en[:], neg[:], Act.Exp)
                    pos = kpool.tile([P, HID], dt.float32, tag="pos")
                    nc.vector.tensor_scalar_max(pos[:], S[:], 0.0)
                    eadd = kpool.tile([P, HID], dt.float32, tag="eadd")
                    nc.vector.tensor_tensor(out=eadd[:], in0=en[:], in1=pos[:],
                                            op=Alu.add)
                    if layer == 1:
                        ebf = kpool.tile([P, HID], dt.bfloat16, tag="ebf")
                        nc.vector.tensor_scalar_add(ebf[:], eadd[:], -1.0)
                        nc.sync.dma_start(out=emb_loc[n0t:n0t + nn_t, :],
                                          in_=ebf[:nn_t, :])
                    else:
                        nc.vector.tensor_scalar_add(
                            ebuf[:, t * HID:(t + 1) * HID], eadd[:], -1.0)
                if layer == 2:
                    for t in range(T):
                        nn_t = min(P, nloc - t * P)
                        nc.tensor.matmul(gps[0:1, :], lhsT=ones_col[:nn_t, :],
                                         rhs=ebuf[:nn_t, t * HID:(t + 1) * HID],
                                         start=(t == 0), stop=(t == T - 1))
                return gps

            # ---------------- main flow ----------------
            def zero_out_vec():
                z = kpool.tile([1, HID], dt.float32, tag="gout", name="z")
                nc.gpsimd.memset(z[:], 0.0)
                nc.sync.dma_start(out=out_vec[:], in_=z[:])

            def flow():
                project(xbf, w1s, K1, table1)
                if stages >= 2:
                    edges(table1, layer=1)
                if stages >= 3:
                    nc.gpsimd.collective_compute(
                        "AllGather", Alu.bypass,
                        ins=[emb_loc[:]], outs=[emb_all[:]],
                        replica_groups=[list(range(N_CORES))])
                if stages >= 4:
                    project(emb_all, w2s, 1, table2)
                if stages == 5:
                    gps = edges(table2, layer=2)
                    gout = kpool.tile([1, HID], dt.float32, tag="gout")
                    nc.vector.tensor_copy(gout[:], gps[:])
                    nc.sync.dma_start(out=out_vec[:], in_=gout[:])
                else:
                    zero_out_vec()

            for _it in range(iters):
                flow()

    nc.compile()
    return nc


# ----------------------------------------------------------------------------
# top-level kernel
# ----------------------------------------------------------------------------

_CACHE = {}


def _run_device(in_maps, meta):
    from concourse.bass_utils import run_bass_kernel_spmd
    key = "prog"
    if key not in _CACHE:
        _CACHE[key] = build_program(meta)
    nc = _CACHE[key]
    res = run_bass_kernel_spmd(nc, in_maps, core_ids=list(range(N_CORES)))
    return res


def host_finish(partials, ln_g, ln_b, Wl1, bl1, Wl2, bl2, Wl3, bl3):
    g = partials.sum(axis=0) / np.float64(N_NODES)  # [HID]
    mu = g.mean()
    var = ((g - mu) ** 2).mean()
    gn = (g - mu) / np.sqrt(var + 1e-5) * ln_g + ln_b
    x = Wl1 @ gn + bl1
    x = np.maximum(x, 0.01 * x)
    x = Wl2 @ x + bl2
    x = np.maximum(x, 0.01 * x)
    x = Wl3 @ x + bl3
    return np.maximum(x, 0.0).astype(np.float32)


def kernel(node_features, edge_src, edge_dst, W1, a1, W2, a2,
           ln_g, ln_b, Wl1, bl1, Wl2, bl2, Wl3, bl3):
    node_features = np.asarray(node_features, dtype=np.float32)
    edge_src = np.asarray(edge_src, dtype=np.int32)
    edge_dst = np.asarray(edge_dst, dtype=np.int32)
    in_maps, meta = host_prep(node_features, edge_src, edge_dst,
                              np.asarray(W1, np.float32), np.asarray(a1, np.float32),
                              np.asarray(W2, np.float32), np.asarray(a2, np.float32))
    res = _run_device(in_maps, meta)
    partials = np.stack([res.results[c]["out_vec"][0] for c in range(N_CORES)])
    return host_finish(partials.astype(np.float64),
                       np.asarray(ln_g, np.float64), np.asarray(ln_b, np.float64),
                       np.asarray(Wl1, np.float64), np.asarray(bl1, np.float64),
                       np.asarray(Wl2, np.float64), np.asarray(bl2, np.float64),
                       np.asarray(Wl3, np.float64), np.asarray(bl3, np.float64))
